# revision 3
# baseline (speedup 1.0000x reference)
"""Trainium2 Bass kernel for nn_Attention_57166014709861.

8-batch image attention (B=8, C=384, h=8, d=48, HW=1024), data-parallel:
one batch image per NeuronCore, weights broadcast, host-side gather.

Per-core pipeline (all matmuls float32r, 1 cycle/row):
  qkv:   q,k packed [d,seq] head-pair tiles (2 heads per 128 partitions,
         offsets 0/64); v computed transposed as vT [seq, packed-c] with a
         ones column per head (denominator rides the av matmul for free).
  attn:  per head: sT[y,x] = k_h^T q_h (scale folded into wq host-side),
         exp on ACT (no max subtraction: |s| <~ 6 so fp32 exp is exact
         enough), av accumulates u'[c,x] over y tiles in PSUM; softmax
         denominator lands in psum row 64; normalize u via reciprocal +
         K=1 broadcast matmul.
  proj:  w_projT packed on K (zero rows in padding), bias added during
         psum->sbuf copy, DMA out.
"""

import sys

if "/opt/trn_rl_repo" not in sys.path:
    sys.path.insert(0, "/opt/trn_rl_repo")

import numpy as np

import concourse.bass as bass
import concourse.mybir as mybir
import concourse.tile as tile
from concourse import bacc
from concourse.bass_utils import run_bass_kernel_spmd

DIM = 384
HEADS = 8
DH = 48
SEQ = 1024
P = 128
NCORES = 8
VP = 80  # packed v columns per head: 48 data + 16 zero + ones at 64 + 15 zero

F32 = mybir.dt.float32
F32R = mybir.dt.float32r
BF16 = mybir.dt.bfloat16
EXP = mybir.ActivationFunctionType.Exp
ADD = mybir.AluOpType.add
MULT = mybir.AluOpType.mult

_NC_CACHE = {}


def _emit(tc, nc, x_d, wq_d, wk_d, wv_d, wp_d, b_d, out_d):
    with (
        tc.tile_pool(name="const", bufs=1) as constp,
        tc.tile_pool(name="weights", bufs=1) as wpool,
        tc.tile_pool(name="data", bufs=1) as data,
        tc.tile_pool(name="ptile", bufs=4) as ppool,
        tc.tile_pool(name="bcpool", bufs=2) as bcpool,
        tc.tile_pool(name="rpool", bufs=2) as rpool,
        tc.tile_pool(name="opool", bufs=2) as opool,
        tc.tile_pool(name="ps_a", bufs=2, space="PSUM") as ps_a,
        tc.tile_pool(name="ps_av", bufs=1, space="PSUM") as ps_av,
        tc.tile_pool(name="ps_bc", bufs=1, space="PSUM") as ps_bc,
    ):
        # ---- loads ----
        x_sb = data.tile([P, 3, SEQ], F32R, tag="x")
        nc.sync.dma_start(x_sb[:], x_d.ap().rearrange("(o p) f -> p o f", p=P))
        wq_sb = wpool.tile([P, 3, 512], F32R, tag="wq")
        nc.sync.dma_start(wq_sb[:], wq_d.ap().rearrange("(o p) f -> p o f", p=P))
        wk_sb = wpool.tile([P, 3, 512], F32R, tag="wk")
        nc.sync.dma_start(wk_sb[:], wk_d.ap().rearrange("(o p) f -> p o f", p=P))
        wv_sb = wpool.tile([P, 3, HEADS * VP], F32R, tag="wv")
        nc.sync.dma_start(wv_sb[:], wv_d.ap().rearrange("(o p) f -> p o f", p=P))
        wp_sb = wpool.tile([P, 4, DIM], F32R, tag="wp")
        nc.sync.dma_start(wp_sb[:], wp_d.ap().rearrange("(o p) f -> p o f", p=P))
        bias_sb = constp.tile([P, 3], F32, tag="bias")
        nc.sync.dma_start(bias_sb[:], b_d.ap())
        ones_sb = constp.tile([1, 64], BF16, tag="ones")
        nc.gpsimd.memset(ones_sb[:], 1.0)
        zb_sb = constp.tile([P, 1], F32, tag="zb")
        nc.gpsimd.memset(zb_sb[:], 0.0)

        # ---- qkv ----
        q_sb = data.tile([P, 4, SEQ], F32R, tag="q")
        k_sb = data.tile([P, 4, SEQ], F32R, tag="k")
        # vT[y, packed-c]: y-tile-major; free dims [yt-free? no: [8 heads, VP]]
        vT_sb = data.tile([P, 8, HEADS, VP], BF16, tag="vT")

        for dst, w in ((q_sb, wq_sb), (k_sb, wk_sb)):
            for t in range(4):
                ps = ps_a.tile([P, SEQ], F32, tag="ps")
                for j in range(2):
                    for ko in range(3):
                        nc.tensor.matmul(
                            ps[:, j * 512 : (j + 1) * 512],
                            lhsT=w[:, ko, t * 128 : (t + 1) * 128],
                            rhs=x_sb[:, ko, j * 512 : (j + 1) * 512],
                            start=(ko == 0),
                            stop=(ko == 2),
                        )
                nc.vector.tensor_copy(dst[:, t, :], ps[:])

        for yt in range(8):
            ps = ps_a.tile([P, SEQ], F32, tag="ps")
            for j in range(2):  # HEADS*VP = 640 = 512 + 128
                lo, hi = j * 512, min((j + 1) * 512, HEADS * VP)
                for ko in range(3):
                    nc.tensor.matmul(
                        ps[:, lo:hi],
                        lhsT=x_sb[:, ko, yt * 128 : (yt + 1) * 128],
                        rhs=wv_sb[:, ko, lo:hi],
                        start=(ko == 0),
                        stop=(ko == 2),
                    )
            nc.vector.tensor_copy(
                vT_sb[:, yt, :, :],
                ps[:, 0 : HEADS * VP].rearrange("p (h v) -> p h v", h=HEADS),
            )
            # ones column for the softmax denominator
            nc.gpsimd.memset(vT_sb[:, yt, :, 64:65], 1.0)

        # ---- attention (per head) ----
        u_sb = data.tile([P, 4, SEQ], F32R, tag="u")
        for h in range(HEADS):
            t, s = h // 2, h % 2
            po = s * 64
            av_ps = ps_av.tile([P, SEQ], F32, tag="av")
            for yt in range(8):
                sT_ps = ps_a.tile([P, SEQ], F32, tag="ps")
                for j in range(2):
                    nc.tensor.matmul(
                        sT_ps[:, j * 512 : (j + 1) * 512],
                        lhsT=k_sb[po : po + 48, t, yt * 128 : (yt + 1) * 128],
                        rhs=q_sb[po : po + 48, t, j * 512 : (j + 1) * 512],
                        start=True,
                        stop=True,
                    )
                p_sb = ppool.tile([P, SEQ], BF16, tag="p")
                nc.scalar.activation(p_sb[:], sT_ps[:], EXP, bias=zb_sb[:])
                for j in range(2):
                    nc.tensor.matmul(
                        av_ps[0:VP, j * 512 : (j + 1) * 512],
                        lhsT=vT_sb[:, yt, h, :],
                        rhs=p_sb[:, j * 512 : (j + 1) * 512],
                        start=(yt == 0),
                        stop=(yt == 7),
                    )
            # denominator is psum row 64; normalize u = av[0:64] * (1/denom)
            recip_sb = rpool.tile([1, SEQ], BF16, tag="recip")
            with nc.allow_low_precision(reason="softmax denom reciprocal to bf16"):
                nc.vector.reciprocal(recip_sb[:], av_ps[64:65, :])
            bc_ps = ps_bc.tile([64, SEQ], F32, tag="bc")
            for j in range(2):
                nc.tensor.matmul(
                    bc_ps[:, j * 512 : (j + 1) * 512],
                    lhsT=ones_sb[:],
                    rhs=recip_sb[:, j * 512 : (j + 1) * 512],
                    start=True,
                    stop=True,
                )
            bc_sb = bcpool.tile([64, SEQ], F32, tag="bcs")
            nc.vector.tensor_copy(bc_sb[:], bc_ps[:])
            nc.vector.tensor_tensor(
                u_sb[po : po + 64, t, :], av_ps[0:64, :], bc_sb[:], MULT
            )

        # ---- proj ----
        for ot in range(3):
            ps = ps_a.tile([P, SEQ], F32, tag="ps")
            for j in range(2):
                for kt in range(4):
                    nc.tensor.matmul(
                        ps[:, j * 512 : (j + 1) * 512],
                        lhsT=wp_sb[:, kt, ot * 128 : (ot + 1) * 128],
                        rhs=u_sb[:, kt, j * 512 : (j + 1) * 512],
                        start=(kt == 0),
                        stop=(kt == 3),
                    )
            o_sb = opool.tile([P, SEQ], F32, tag="o")
            nc.vector.tensor_scalar(o_sb[:], ps[:], bias_sb[:, ot : ot + 1], None, ADD)
            nc.sync.dma_start(out_d.ap()[ot * 128 : (ot + 1) * 128, :], o_sb[:])


def build_nc():
    nc = bacc.Bacc("TRN2", target_bir_lowering=False, debug=False, num_devices=NCORES)
    x_d = nc.dram_tensor("x", [DIM, SEQ], F32R, kind="ExternalInput")
    wq_d = nc.dram_tensor("wq", [DIM, 512], F32R, kind="ExternalInput")
    wk_d = nc.dram_tensor("wk", [DIM, 512], F32R, kind="ExternalInput")
    wv_d = nc.dram_tensor("wv", [DIM, HEADS * VP], F32R, kind="ExternalInput")
    wp_d = nc.dram_tensor("wp", [512, DIM], F32R, kind="ExternalInput")
    b_d = nc.dram_tensor("bias", [P, 3], F32, kind="ExternalInput")
    out_d = nc.dram_tensor("out", [DIM, SEQ], F32, kind="ExternalOutput")

    with tile.TileContext(nc) as tc:
        _emit(tc, nc, x_d, wq_d, wk_d, wv_d, wp_d, b_d, out_d)
    nc.compile()
    return nc


def pack_inputs(x, w_qkv, w_proj, b_proj):
    """Host-side weight packing. Returns per-core input maps."""
    x = np.asarray(x, np.float32)
    w_qkv = np.asarray(w_qkv, np.float32)
    w_proj = np.asarray(w_proj, np.float32)
    b_proj = np.asarray(b_proj, np.float32)
    scale = DH ** -0.5
    w_q, w_k, w_v = w_qkv[0:DIM], w_qkv[DIM : 2 * DIM], w_qkv[2 * DIM :]

    WQ = np.zeros((DIM, 512), np.float32)
    WK = np.zeros((DIM, 512), np.float32)
    WV = np.zeros((DIM, HEADS * VP), np.float32)
    WP = np.zeros((512, DIM), np.float32)
    for h in range(HEADS):
        col = (h // 2) * 128 + (h % 2) * 64
        WQ[:, col : col + DH] = (w_q[h * DH : (h + 1) * DH] * scale).T
        WK[:, col : col + DH] = w_k[h * DH : (h + 1) * DH].T
        WV[:, h * VP : h * VP + DH] = w_v[h * DH : (h + 1) * DH].T
        WP[col : col + DH, :] = w_proj[:, h * DH : (h + 1) * DH].T
    BIAS = np.ascontiguousarray(b_proj.reshape(3, P).T)

    in_maps = []
    for b in range(NCORES):
        in_maps.append(
            {
                "x": np.ascontiguousarray(x[b].reshape(DIM, SEQ)),
                "wq": WQ,
                "wk": WK,
                "wv": WV,
                "wp": WP,
                "bias": BIAS,
            }
        )
    return in_maps


def run(in_maps, trace=False):
    if "nc" not in _NC_CACHE:
        _NC_CACHE["nc"] = build_nc()
    nc = _NC_CACHE["nc"]
    res = run_bass_kernel_spmd(
        nc, in_maps, core_ids=list(range(NCORES)), trace=trace
    )
    out = np.stack([res.results[i]["out"] for i in range(NCORES)])
    return out.reshape(NCORES, DIM, 32, 32), res


def kernel(x, w_qkv, w_proj, b_proj):
    out, _ = run(pack_inputs(x, w_qkv, w_proj, b_proj))
    return out


# revision 6
# speedup vs baseline: 1.1919x; 1.1919x over previous
"""Trainium2 Bass kernel for nn_Attention_57166014709861.

8-batch image attention (B=8, C=384, h=8, d=48, HW=1024), data-parallel:
one batch image per NeuronCore, weights broadcast, host-side gather.

Per-core pipeline (all matmuls float32r, 1 cycle/row):
  qkv:   q,k packed [d,seq] head-pair tiles (2 heads per 128 partitions,
         offsets 0/64); v computed transposed as vT [seq, packed-c] with a
         ones column per head (denominator rides the av matmul for free).
  attn:  per head: sT[y,x] = k_h^T q_h (scale folded into wq host-side),
         exp on ACT (no max subtraction: |s| <~ 6 so fp32 exp is exact
         enough), av accumulates u'[c,x] over y tiles in PSUM; softmax
         denominator lands in psum row 64; normalize u via reciprocal +
         K=1 broadcast matmul.
  proj:  w_projT packed on K (zero rows in padding), bias added during
         psum->sbuf copy, DMA out.
"""

import sys

if "/opt/trn_rl_repo" not in sys.path:
    sys.path.insert(0, "/opt/trn_rl_repo")

import numpy as np

import concourse.bass as bass
import concourse.mybir as mybir
import concourse.tile as tile
from concourse import bacc
from concourse.bass_utils import run_bass_kernel_spmd

DIM = 384
HEADS = 8
DH = 48
SEQ = 1024
P = 128
NCORES = 8
VP = 80  # packed v columns per head: 48 data + 16 zero + ones at 64 + 15 zero

F32 = mybir.dt.float32
F32R = mybir.dt.float32r
BF16 = mybir.dt.bfloat16
EXP = mybir.ActivationFunctionType.Exp
ADD = mybir.AluOpType.add
MULT = mybir.AluOpType.mult

_NC_CACHE = {}


def _emit(tc, nc, x_d, wq_d, wk_d, wv_d, wp_d, b_d, out_d):
    with (
        tc.tile_pool(name="const", bufs=1) as constp,
        tc.tile_pool(name="weights", bufs=1) as wpool,
        tc.tile_pool(name="data", bufs=1) as data,
        tc.tile_pool(name="ptile", bufs=4) as ppool,
        tc.tile_pool(name="bcpool", bufs=2) as bcpool,
        tc.tile_pool(name="rpool", bufs=2) as rpool,
        tc.tile_pool(name="opool", bufs=2) as opool,
        tc.tile_pool(name="ps_a", bufs=2, space="PSUM") as ps_a,
        tc.tile_pool(name="ps_av", bufs=2, space="PSUM") as ps_av,
        tc.tile_pool(name="dram", bufs=2, space="DRAM") as drampool,
    ):
        # ---- loads ----
        x_sb = data.tile([P, 3, SEQ], F32R, tag="x")
        nc.sync.dma_start(x_sb[:], x_d.ap().rearrange("(o p) f -> p o f", p=P))
        wq_sb = wpool.tile([P, 3, 512], F32R, tag="wq")
        nc.sync.dma_start(wq_sb[:], wq_d.ap().rearrange("(o p) f -> p o f", p=P))
        wk_sb = wpool.tile([P, 3, 512], F32R, tag="wk")
        nc.sync.dma_start(wk_sb[:], wk_d.ap().rearrange("(o p) f -> p o f", p=P))
        wv_sb = wpool.tile([P, 3, HEADS * VP], F32R, tag="wv")
        nc.sync.dma_start(wv_sb[:], wv_d.ap().rearrange("(o p) f -> p o f", p=P))
        wp_sb = wpool.tile([P, 4, DIM], F32R, tag="wp")
        nc.sync.dma_start(wp_sb[:], wp_d.ap().rearrange("(o p) f -> p o f", p=P))
        bias_sb = constp.tile([P, 3], F32, tag="bias")
        nc.sync.dma_start(bias_sb[:], b_d.ap())
        ones_sb = constp.tile([1, 64], BF16, tag="ones")
        nc.gpsimd.memset(ones_sb[:], 1.0)
        zb_sb = constp.tile([P, 1], F32, tag="zb")
        nc.gpsimd.memset(zb_sb[:], 0.0)

        # ---- qkv ----
        q_sb = data.tile([P, 4, SEQ], F32R, tag="q")
        k_sb = data.tile([P, 4, SEQ], F32R, tag="k")
        # vT[y, packed-c]: y-tile-major; free dims [yt-free? no: [8 heads, VP]]
        vT_sb = data.tile([P, 8, HEADS, VP], BF16, tag="vT")

        for dst, w in ((q_sb, wq_sb), (k_sb, wk_sb)):
            for t in range(4):
                ps = ps_a.tile([P, SEQ], F32, tag="ps")
                for j in range(2):
                    for ko in range(3):
                        nc.tensor.matmul(
                            ps[:, j * 512 : (j + 1) * 512],
                            lhsT=w[:, ko, t * 128 : (t + 1) * 128],
                            rhs=x_sb[:, ko, j * 512 : (j + 1) * 512],
                            start=(ko == 0),
                            stop=(ko == 2),
                        )
                nc.vector.tensor_copy(dst[:, t, :], ps[:])

        for yt in range(8):
            ps = ps_a.tile([P, SEQ], F32, tag="ps")
            for j in range(2):  # HEADS*VP = 640 = 512 + 128
                lo, hi = j * 512, min((j + 1) * 512, HEADS * VP)
                for ko in range(3):
                    nc.tensor.matmul(
                        ps[:, lo:hi],
                        lhsT=x_sb[:, ko, yt * 128 : (yt + 1) * 128],
                        rhs=wv_sb[:, ko, lo:hi],
                        start=(ko == 0),
                        stop=(ko == 2),
                    )
            nc.vector.tensor_copy(
                vT_sb[:, yt, :, :],
                ps[:, 0 : HEADS * VP].rearrange("p (h v) -> p h v", h=HEADS),
            )
            # ones column for the softmax denominator
            nc.gpsimd.memset(vT_sb[:, yt, :, 64:65], 1.0)

        # ---- attention (per head) ----
        u_sb = data.tile([P, 4, SEQ], F32R, tag="u")

        def epilogue(h, av_ps):
            # denominator = psum row 64. DMA-transpose it to [128, 8] so the
            # reciprocal runs on 128 lanes (free-major [1,1024] costs 6.5us),
            # DMA back to free-major, broadcast on GpSimd, multiply on DVE.
            # The whole chain avoids the PE.
            t, s = h // 2, h % 2
            po = s * 64
            den_row = rpool.tile([1, SEQ], F32, tag="denrow")
            nc.vector.tensor_copy(den_row[:], av_ps[64:65, :])
            den_dram = drampool.tile([SEQ], F32, tag="den")
            nc.sync.dma_start(den_dram[:], den_row[:])
            den_pm = rpool.tile([P, 8], F32, tag="denpm")
            nc.sync.dma_start(den_pm[:], den_dram[:].rearrange("(p f) -> p f", p=P))
            rec_pm = rpool.tile([P, 8], BF16, tag="recpm")
            with nc.allow_low_precision(reason="softmax denom reciprocal to bf16"):
                nc.vector.reciprocal(rec_pm[:], den_pm[:])
            rec_dram = drampool.tile([SEQ], BF16, tag="rec")
            nc.sync.dma_start(rec_dram[:], rec_pm[:])
            rec_sb = rpool.tile([1, SEQ], BF16, tag="rec1")
            nc.sync.dma_start(rec_sb[:], rec_dram[:].rearrange("(o f) -> o f", o=1))
            bc_sb = bcpool.tile([64, SEQ], BF16, tag="bcs")
            nc.gpsimd.partition_broadcast(bc_sb[:], rec_sb[:])
            nc.vector.tensor_tensor(
                u_sb[po : po + 64, t, :], av_ps[0:64, :], bc_sb[:], MULT
            )

        pending = None
        for h in range(HEADS):
            t, s = h // 2, h % 2
            po = s * 64
            av_ps = ps_av.tile([P, SEQ], F32, tag="av")
            for yt in range(8):
                sT_ps = ps_a.tile([P, SEQ], F32, tag="ps")
                for j in range(2):
                    nc.tensor.matmul(
                        sT_ps[:, j * 512 : (j + 1) * 512],
                        lhsT=k_sb[po : po + 48, t, yt * 128 : (yt + 1) * 128],
                        rhs=q_sb[po : po + 48, t, j * 512 : (j + 1) * 512],
                        start=True,
                        stop=True,
                    )
                p_sb = ppool.tile([P, SEQ], BF16, tag="p")
                nc.scalar.activation(p_sb[:], sT_ps[:], EXP, bias=zb_sb[:])
                for j in range(2):
                    nc.tensor.matmul(
                        av_ps[0:VP, j * 512 : (j + 1) * 512],
                        lhsT=vT_sb[:, yt, h, :],
                        rhs=p_sb[:, j * 512 : (j + 1) * 512],
                        start=(yt == 0),
                        stop=(yt == 7),
                    )
                if yt == 1 and pending is not None:
                    epilogue(*pending)
                    pending = None
            pending = (h, av_ps)
        epilogue(*pending)

        # ---- proj ----
        for ot in range(3):
            ps = ps_a.tile([P, SEQ], F32, tag="ps")
            for j in range(2):
                for kt in range(4):
                    nc.tensor.matmul(
                        ps[:, j * 512 : (j + 1) * 512],
                        lhsT=wp_sb[:, kt, ot * 128 : (ot + 1) * 128],
                        rhs=u_sb[:, kt, j * 512 : (j + 1) * 512],
                        start=(kt == 0),
                        stop=(kt == 3),
                    )
            o_sb = opool.tile([P, SEQ], F32, tag="o")
            nc.vector.tensor_scalar(o_sb[:], ps[:], bias_sb[:, ot : ot + 1], None, ADD)
            nc.sync.dma_start(out_d.ap()[ot * 128 : (ot + 1) * 128, :], o_sb[:])


def build_nc():
    nc = bacc.Bacc("TRN2", target_bir_lowering=False, debug=False, num_devices=NCORES)
    x_d = nc.dram_tensor("x", [DIM, SEQ], F32R, kind="ExternalInput")
    wq_d = nc.dram_tensor("wq", [DIM, 512], F32R, kind="ExternalInput")
    wk_d = nc.dram_tensor("wk", [DIM, 512], F32R, kind="ExternalInput")
    wv_d = nc.dram_tensor("wv", [DIM, HEADS * VP], F32R, kind="ExternalInput")
    wp_d = nc.dram_tensor("wp", [512, DIM], F32R, kind="ExternalInput")
    b_d = nc.dram_tensor("bias", [P, 3], F32, kind="ExternalInput")
    out_d = nc.dram_tensor("out", [DIM, SEQ], F32, kind="ExternalOutput")

    with tile.TileContext(nc) as tc:
        _emit(tc, nc, x_d, wq_d, wk_d, wv_d, wp_d, b_d, out_d)
    nc.compile()
    return nc


def pack_inputs(x, w_qkv, w_proj, b_proj):
    """Host-side weight packing. Returns per-core input maps."""
    x = np.asarray(x, np.float32)
    w_qkv = np.asarray(w_qkv, np.float32)
    w_proj = np.asarray(w_proj, np.float32)
    b_proj = np.asarray(b_proj, np.float32)
    scale = DH ** -0.5
    w_q, w_k, w_v = w_qkv[0:DIM], w_qkv[DIM : 2 * DIM], w_qkv[2 * DIM :]

    WQ = np.zeros((DIM, 512), np.float32)
    WK = np.zeros((DIM, 512), np.float32)
    WV = np.zeros((DIM, HEADS * VP), np.float32)
    WP = np.zeros((512, DIM), np.float32)
    for h in range(HEADS):
        col = (h // 2) * 128 + (h % 2) * 64
        WQ[:, col : col + DH] = (w_q[h * DH : (h + 1) * DH] * scale).T
        WK[:, col : col + DH] = w_k[h * DH : (h + 1) * DH].T
        WV[:, h * VP : h * VP + DH] = w_v[h * DH : (h + 1) * DH].T
        WP[col : col + DH, :] = w_proj[:, h * DH : (h + 1) * DH].T
    BIAS = np.ascontiguousarray(b_proj.reshape(3, P).T)

    in_maps = []
    for b in range(NCORES):
        in_maps.append(
            {
                "x": np.ascontiguousarray(x[b].reshape(DIM, SEQ)),
                "wq": WQ,
                "wk": WK,
                "wv": WV,
                "wp": WP,
                "bias": BIAS,
            }
        )
    return in_maps


def run(in_maps, trace=False):
    if "nc" not in _NC_CACHE:
        _NC_CACHE["nc"] = build_nc()
    nc = _NC_CACHE["nc"]
    res = run_bass_kernel_spmd(
        nc, in_maps, core_ids=list(range(NCORES)), trace=trace
    )
    out = np.stack([res.results[i]["out"] for i in range(NCORES)])
    return out.reshape(NCORES, DIM, 32, 32), res


def kernel(x, w_qkv, w_proj, b_proj):
    out, _ = run(pack_inputs(x, w_qkv, w_proj, b_proj))
    return out
